# revision 50
# baseline (speedup 1.0000x reference)
"""Trainium2 Bass kernel for nn_KronQRLinearLayer3_cayley.

Computes out = x @ R @ W^T where R = kron(kron(q1, q2), q3) and the q_i are
Cayley transforms (orthogonal) of the tiny kron_i inputs.

Strategy (per spec sharding_hint):
  - Data-parallel over the batch dim: core b gets x[b] = [4096, 1280] tokens.
  - kron factors + W replicated on every core.
  - The main GEMM runs as fp8e4m3 DoubleRow matmuls (0.5 cyc/out-row,
    K=256/instr). Accuracy is recovered with a same-scale residual 3-split:
        x @ M  ~=  x8 @ M8 + xr8 @ M8 + x8 @ Mr8
    where x8 = fp8(x*s), xr8 = fp8(x*s - x8) (same scale, so all 15 partial
    matmuls accumulate in a single PSUM group). End-to-end rel err ~3.6e-3.
  - Host pre-transposes and DR-packs x so the device does ZERO transposes:
    x8[p, kp, jj, t] = fp8(16 * x[t, 256*kp + 128*jj + p]).
  - On device, per core:
      1. Cayley q_i^T via transpose-free Newton-Schulz inverse iteration on
         one block-diagonal [100,100] packing (q3@0, q2@64, q1@72), with a
         depth-2 critical cycle (z = 2I - Y on DVE, Y' = Yz on PE).
      2. R^T tiles [128, 1280] in bf16, one DVE broadcast-multiply each,
         from K12T = q1T (x) q2T and q3T selection-matrix gathers.
      3. M = R @ W^T as a plain bf16 GEMM (300 matmuls, 128k cycles; rhs
         W^T supplied by the host in bf16), PSUM output quantized+packed
         into the fp8 DR pair m8/mr8. The first 6 PSUM groups are emitted
         k-major so the PE marches with rt16 tile production.
      4. Main GEMM out = x @ M as 3-split fp8 DR (2400 matmuls, 307k
         cycles), PSUM -> bf16 out tiles (scale 1/8192) -> DRAM.

Self-contained: hardcodes all shapes; host does sharding, scaling,
transpose/packing, fp8/bf16 conversion, and the final bf16->f32 gather.
"""

import numpy as np
import ml_dtypes

B, S, D = 8, 4096, 1280
K1, K2, K3 = 4, 8, 40
G12 = K1 * K2  # 32
NT = S // 128           # 32 token tiles per core
KP = D // 256           # 5 k-pairs (DoubleRow contracts 256 per matmul)
NOC = D // 256          # 5 output chunks of 256
NEWTON_ITERS = 8
# 1/s scale for Newton X0 = B^T/s; convergence needs lam_max(I + S S^T)/s < 2,
# fastest when s ~= (lam_min + lam_max)/2 (err0 = max|1 - lam/s|).
# Measured lam_max: 4.4 / 9.1 / 71  ->  err0 = 0.75 / 0.875 / 0.972;
# 0.972^(2^9) ~= 5e-7.
INV_S = {4: 1.0 / 4.0, 8: 1.0 / 8.0, 40: 1.0 / 36.0}
# fp8 pre-scales (keep quantized values out of subnormal range; powers of 2)
SX = 16.0     # x * SX
SM = 512.0    # M * SM
F8 = ml_dtypes.float8_e4m3

_CACHE = {}


def _host_constants():
    # sel40t[:, k*128+p] one-hot over r=(128k+p)%40  -> lhsT [40, 1280]
    sel40t = np.zeros((K3, D), np.float32)
    sel32t = np.zeros((G12, D), np.float32)
    j = np.arange(D)
    sel40t[j % K3, j] = 1.0
    sel32t[j // K3, j] = 1.0
    # mini selections for K12T build, placed at the partitions where the
    # q2/q1 blocks of qt_all live (64 / 72) so no realignment DMA is needed
    # (SBUF AP base partition must be 0/32/64; both gathers read from base 64).
    # cols 0:32 select q2 rows (64+b'), cols 32:64 select q1 rows (72+a').
    sel48 = np.zeros((128, 2 * G12), np.float32)
    p = np.arange(G12)
    sel48[64 + p % K2, p] = 1.0
    sel48[72 + p // K2, G12 + p] = 1.0
    return {"sel40t": sel40t, "sel32t": sel32t, "sel48": sel48}


def _newton_pack(kron_1, kron_2, kron_3):
    """[100, 500] = [ball | bnall | x0 | v0 | twoiall] for the block-diagonal
    Cayley packing (q3@0, q2@64, q1@96). Pure elementwise input prep; the
    Newton-Schulz inverse iteration itself runs on device."""
    NP_ = 100
    iall = np.zeros((NP_, NP_), np.float32)
    s05 = np.zeros((NP_, NP_), np.float32)
    svec = np.ones((NP_, 1), np.float32)
    for a, n, off in ((kron_3, K3, 0), (kron_2, K2, 64), (kron_1, K1, 72)):
        iall[off:off + n, off:off + n] = np.eye(n)
        s05[off:off + n, off:off + n] = 0.5 * (a - a.T)
        svec[off:off + n] = INV_S[n]
    ball = iall + s05
    bnall = iall - s05
    return np.ascontiguousarray(np.concatenate(
        [ball, bnall, bnall * svec, ball * svec, 2.0 * iall],
        axis=1).astype(np.float32))


def _pack_dr(a_t, scale):
    """[D, C] (rows j, any cols) -> (hi, lo) fp8 DR packs [128, KP, 2, C].

    hi[p, kp, jj, c] = fp8(scale * a_t[256*kp + 128*jj + p, c]);
    lo = fp8(scale * a_t - hi)  (same scale -> shared PSUM group).
    """
    c = a_t.shape[1]
    sc = (a_t * scale).astype(np.float32)
    hi = sc.astype(F8)
    lo = (sc - hi.astype(np.float32)).astype(F8)
    hi = np.ascontiguousarray(hi.reshape(KP, 2, 128, c).transpose(2, 0, 1, 3))
    lo = np.ascontiguousarray(lo.reshape(KP, 2, 128, c).transpose(2, 0, 1, 3))
    return hi, lo


def build_program():
    """Build the single-core Bass/Tile program (shared SPMD across 8 cores)."""
    import concourse.bacc as bacc
    import concourse.mybir as mybir
    import concourse.tile as tile

    f32 = mybir.dt.float32
    f8 = mybir.dt.float8e4
    bf16 = mybir.dt.bfloat16
    DR = mybir.MatmulPerfMode.DoubleRow

    nc = bacc.Bacc("TRN2", target_bir_lowering=False, debug=False)

    x8_d = nc.dram_tensor("x8", [128, KP, 2, S], f8, kind="ExternalInput").ap()
    xr8_d = nc.dram_tensor("xr8", [128, KP, 2, S], f8, kind="ExternalInput").ap()
    w16_d = nc.dram_tensor("wt16", [128, 2 * KP, D], bf16,
                           kind="ExternalInput").ap()
    np_d = nc.dram_tensor("npack", [100, 500], f32, kind="ExternalInput").ap()
    c_d = {}
    for name, arr in _host_constants().items():
        c_d[name] = nc.dram_tensor(name, list(arr.shape), f32, kind="ExternalInput").ap()
    out_d = nc.dram_tensor("out", [S, D], bf16, kind="ExternalOutput").ap()

    from contextlib import ExitStack

    with tile.TileContext(nc) as tc, ExitStack() as stack:
        # ---- persistent pools -------------------------------------------
        # DMA issue order is the DMA-engine drain order: Newton pack first,
        # then the small selection consts, then W packs, then bulk x packs.
        cpool = stack.enter_context(tc.tile_pool(name="consts", bufs=1))
        npk = cpool.tile([100, 500], f32, name="npack")
        nc.sync.dma_start(npk[:, :], np_d[:, :])
        sel48 = cpool.tile([128, 2 * G12], f32, name="sel48")
        nc.sync.dma_start(sel48[:, :], c_d["sel48"][:, :])
        sel40t = cpool.tile([K3, D], f32, name="sel40t")
        nc.sync.dma_start(sel40t[:, :], c_d["sel40t"][:, :])
        sel32t = cpool.tile([G12, D], f32, name="sel32t")
        nc.sync.dma_start(sel32t[:, :], c_d["sel32t"][:, :])

        xpool = stack.enter_context(tc.tile_pool(name="xres", bufs=1))
        x8_sb = xpool.tile([128, KP, 2, S], f8, name="x8")
        xr8_sb = xpool.tile([128, KP, 2, S], f8, name="xr8")

        mpool = stack.enter_context(tc.tile_pool(name="mmat", bufs=1))
        m8_sb = mpool.tile([128, KP, 2, D], f8, name="m8")
        mr8_sb = mpool.tile([128, KP, 2, D], f8, name="mr8")

        # ---- prologue: Cayley + R^T + M-GEMM ----------------------------
        with (
            tc.tile_pool(name="prosb", bufs=1) as ppool,
            tc.tile_pool(name="prow", bufs=1) as wpool,
        ):
            # Newton/gather PSUM pool is closed (banks freed) before the
            # M-GEMM pool opens, so the M-GEMM can hold 8 full-bank groups.
            npsum_stack = ExitStack()
            ppsum = npsum_stack.enter_context(
                tc.tile_pool(name="newtpsum", bufs=1, space="PSUM"))
            # --- Cayley: transpose-free Newton-Schulz on one block-diagonal
            #     [100,100] packing (q3@0, q2@64, q1@96). blockdiag x blockdiag
            #     stays blockdiag, so one matmul drives all three factors.
            #     ball/bnall/x0/v0/twoiall come prebuilt in npack. ---
            NP_ = 100
            ball = npk[:, 0:100]
            bnall = npk[:, 100:200]
            twoiall = npk[:, 400:500]

            wt16_sb = wpool.tile([128, 2 * KP, D], bf16, name="wt16")
            nc.sync.dma_start(wt16_sb[:, :, :], w16_d[:, :, :])
            rt16s = [wpool.tile([128, D], bf16, tag=f"rt16_{k}", bufs=1,
                                name=f"rt16_{k}") for k in range(2 * KP)]
            # x streamed in 4 token spans so the main loop can start early
            for sp in range(4):
                t0 = sp * (S // 4)
                nc.sync.dma_start(x8_sb[:, :, :, t0:t0 + S // 4],
                                  x8_d[:, :, :, t0:t0 + S // 4])
                nc.sync.dma_start(xr8_sb[:, :, :, t0:t0 + S // 4],
                                  xr8_d[:, :, :, t0:t0 + S // 4])

            # Iteration tracks Y = B X (psum-to-DVE cycle of depth 2:
            # z = 2I - Y on DVE, Y' = Y z on PE) plus, off the critical
            # cycle, (X, X^T, Y^T) one step behind:
            #   X' = X z   (lhsT = X^T), X'^T = z^T X^T (lhsT = z)
            #   Y'^T = z^T Y^T (lhsT = z);  copies feed the next iteration.
            xcur = npk[:, 200:300]   # X0 (= X0^T: X0 is B^T/s, but we only
            vcur = npk[:, 300:400]   # X0^T = B/s = v0
            y_ps = ppsum.tile([NP_, NP_], f32, tag="cay", bufs=2, name="y_ps")
            nc.tensor.matmul(y_ps[:, :], bnall[:, :], xcur[:, :],
                             start=True, stop=True)  # Y0 = B X0
            yt_sb = ppool.tile([NP_, NP_], f32, tag="yt", bufs=2, name="yt0")
            nc.scalar.copy(yt_sb[:, :], y_ps[:, :])  # Y0^T = Y0? no: see below
            # Y0 = B X0 = B B^T/s is symmetric, so Y0^T = Y0 and the plain
            # copy above is valid; later Y's stay symmetric (Y' = Y(2I-Y)).
            for newton_i in range(NEWTON_ITERS):
                z = ppool.tile([NP_, NP_], f32, tag="z", bufs=2, name="z")
                nc.vector.tensor_sub(z[:, :], twoiall[:, :], y_ps[:, :])
                # critical Y-cycle first so the PE runs it as soon as z lands
                if newton_i < NEWTON_ITERS - 1:
                    yn_ps = ppsum.tile([NP_, NP_], f32, tag="cay", bufs=2,
                                       name="yn_ps")
                    nc.tensor.matmul(yn_ps[:, :], yt_sb[:, :], z[:, :],
                                     start=True, stop=True)  # Y' = Y z
                    ytn = ppool.tile([NP_, NP_], f32, tag="yt", bufs=2, name="ytn")
                    nc.scalar.copy(ytn[:, :], yn_ps[:, :])  # Y' symmetric
                    y_ps_next, yt_sb = yn_ps, ytn
                xn_ps = ppsum.tile([NP_, NP_], f32, tag="cayx", bufs=2, name="xn_ps")
                nc.tensor.matmul(xn_ps[:, :], vcur[:, :], z[:, :],
                                 start=True, stop=True)  # X' = X z
                vn_ps = ppsum.tile([NP_, NP_], f32, tag="cayx", bufs=2, name="vn_ps")
                nc.tensor.matmul(vn_ps[:, :], z[:, :], vcur[:, :],
                                 start=True, stop=True)  # X'^T = z^T X^T
                if newton_i < NEWTON_ITERS - 1:
                    y_ps = y_ps_next
                xn = ppool.tile([NP_, NP_], f32, tag="xv", bufs=2, name="xn")
                nc.vector.tensor_copy(xn[:, :], xn_ps[:, :])
                vn = ppool.tile([NP_, NP_], f32, tag="xv", bufs=2, name="vn")
                nc.scalar.copy(vn[:, :], vn_ps[:, :])
                xcur, vcur = xn, vn

            qt_ps = ppsum.tile([NP_, NP_], f32, tag="cay", bufs=2, name="qt_ps")
            nc.tensor.matmul(qt_ps[:, :], xcur[:, :], ball[:, :],
                             start=True, stop=True)  # qT = X^T B (blockdiag)
            qt_all = ppool.tile([NP_, NP_], f32, name="qt_all")
            nc.vector.tensor_copy(qt_all[:, :], qt_ps[:, :])
            qt3 = qt_all[0:K3, 0:K3]

            # --- K12T = q1T (x) q2T  [32,32];
            #     the q1/q2 blocks are read in place at partitions 64/72 ---
            q1r_ps = ppsum.tile([G12, K1], f32, tag="cay", bufs=2, name="q1r_ps")
            nc.tensor.matmul(q1r_ps[:, :], sel48[64:64 + 12, G12:2 * G12],
                             qt_all[64:64 + 12, 72:72 + K1],
                             start=True, stop=True)
            q1r = ppool.tile([G12, K1], f32, name="q1r")
            nc.vector.tensor_copy(q1r[:, :], q1r_ps[:, :])
            q2r_ps = ppsum.tile([G12, K2], f32, tag="cay", bufs=2, name="q2r_ps")
            nc.tensor.matmul(q2r_ps[:, :], sel48[64:64 + K2, 0:G12],
                             qt_all[64:64 + K2, 64:64 + K2],
                             start=True, stop=True)
            q2r = ppool.tile([G12, K2], f32, name="q2r")
            nc.vector.tensor_copy(q2r[:, :], q2r_ps[:, :])
            k12t = ppool.tile([G12, G12], f32, name="k12t")
            nc.vector.tensor_tensor(
                k12t.rearrange("p (a b) -> p a b", b=K2),
                q1r.unsqueeze(2).broadcast_to([G12, K1, K2]),
                q2r.unsqueeze(1).broadcast_to([G12, K1, K2]),
                op=mybir.AluOpType.mult,
            )

            # --- R^T tiles [128, 1280] bf16: rows j=(g',c'), RT[j,(g,c)] =
            #     K12T[g',g] * q3T[c',c]. One DVE broadcast-mult per tile
            #     writes bf16 directly -- no quantization stream at all. ---
            q3rs, krs = [], []
            for k in range(2 * KP):
                q3r_ps = ppsum.tile([128, K3], f32, tag="cay", bufs=2,
                                    name="q3r_ps")
                nc.tensor.matmul(q3r_ps[:, :], sel40t[:, k * 128:(k + 1) * 128],
                                 qt3, start=True, stop=True)
                q3r = ppool.tile([128, K3], f32, tag=f"q3r{k}", bufs=1,
                                 name="q3r")
                nc.vector.tensor_copy(q3r[:, :], q3r_ps[:, :])
                q3rs.append(q3r)
                kr_ps = ppsum.tile([128, G12], f32, tag="cay", bufs=2,
                                   name="kr_ps")
                nc.tensor.matmul(kr_ps[:, :], sel32t[:, k * 128:(k + 1) * 128],
                                 k12t[:, :], start=True, stop=True)
                kr = ppool.tile([128, G12], f32, tag=f"kr{k}", bufs=1, name="kr")
                nc.scalar.copy(kr[:, :], kr_ps[:, :])
                krs.append(kr)
            for k in range(2 * KP):
                # all mults on DVE: Pool must stay free for the mr8 subs
                # (a clogged Pool queue stalls the mtmp pool rotation)
                nc.vector.tensor_tensor(
                    rt16s[k].rearrange("p (g c) -> p g c", c=K3),
                    krs[k].unsqueeze(2).broadcast_to([128, G12, K3]),
                    q3rs[k].unsqueeze(1).broadcast_to([128, G12, K3]),
                    op=mybir.AluOpType.mult,
                )

            # --- M = R @ W^T as a plain bf16 GEMM (1.0 cyc/row), then
            #     quantize+pack into the fp8 DR packs for the main loop.
            #     The first 8 groups are emitted k-major so they march in
            #     step with the rt16 production instead of the first group
            #     crawling through k serially (which would idle the PE). ---
            npsum_stack.close()
            mg_stack = ExitStack()
            mgps = mg_stack.enter_context(
                tc.tile_pool(name="mgpsum", bufs=1, space="PSUM"))
            M_CHUNKS = [(0, 512), (512, 512), (1024, 256)]
            WAVE = 8
            groups = [(it, o0, cw) for it in range(2 * KP)
                      for (o0, cw) in M_CHUNKS]
            accs, mtmps, done = {}, {}, {}

            def start_group(g):
                accs[g] = mgps.tile([128, 512], f32, tag="mgemm", bufs=WAVE,
                                    name="m_acc")

            def emit_mm(g, k):
                it, o0, cw = g
                nc.tensor.matmul(
                    accs[g][:, :cw],
                    rt16s[k][:, it * 128:(it + 1) * 128],
                    wt16_sb[:, k, o0:o0 + cw],
                    start=(k == 0), stop=(k == 2 * KP - 1),
                )

            def finish_group(g):
                it, o0, cw = g
                if it not in mtmps:
                    mtmps[it] = ppool.tile([128, D], f32, tag="mtmp", bufs=5,
                                           name="mtmp")
                mtmp = mtmps[it]
                nc.scalar.mul(mtmp[:, o0:o0 + cw], accs.pop(g)[:, :cw], SM)
                kp_i, jj_i = it // 2, it % 2
                if it == 2 * KP - 1:
                    # last tile gates the main loop: quantize per chunk
                    sl = slice(o0, o0 + cw)
                    nc.vector.tensor_copy(m8_sb[:, kp_i, jj_i, sl], mtmp[:, sl])
                    nc.gpsimd.tensor_sub(mr8_sb[:, kp_i, jj_i, sl], mtmp[:, sl],
                                         m8_sb[:, kp_i, jj_i, sl])
                else:
                    done[it] = done.get(it, 0) + 1
                    if done[it] == len(M_CHUNKS):
                        nc.vector.tensor_copy(m8_sb[:, kp_i, jj_i, :],
                                              mtmp[:, :])
                        nc.gpsimd.tensor_sub(mr8_sb[:, kp_i, jj_i, :],
                                             mtmp[:, :],
                                             m8_sb[:, kp_i, jj_i, :])

            wave, rest = groups[:WAVE], groups[WAVE:]
            for g in wave:
                start_group(g)
            for k in range(2 * KP):
                for g in wave:
                    emit_mm(g, k)
            for g in wave:
                finish_group(g)
            for g in rest:
                start_group(g)
                for k in range(2 * KP):
                    emit_mm(g, k)
                finish_group(g)
            mg_stack.close()

        # ---- main loop: out = x @ M (3-split fp8 DR) --------------------
        with (
            tc.tile_pool(name="osb", bufs=3) as opool,
            tc.tile_pool(name="mainpsum", bufs=1, space="PSUM") as mpsum,
        ):
            for ti in range(NT):
                o_sb = opool.tile([128, D], bf16, tag="o", name="o_sb")
                for oc in range(NOC):
                    acc = mpsum.tile([128, 256], f32, tag="acc", bufs=6,
                                     name="acc")
                    idx = 0
                    # kp-major: matmuls needing the last-quantized M pack
                    # (kp = KP-1) come last in the accumulation group
                    for kp in range(KP):
                        for lhs, rhs in ((x8_sb, m8_sb), (xr8_sb, m8_sb),
                                         (x8_sb, mr8_sb)):
                            nc.tensor.matmul(
                                acc[:, :],
                                lhs[:, kp, :, ti * 128:(ti + 1) * 128],
                                rhs[:, kp, :, oc * 256:(oc + 1) * 256],
                                start=(idx == 0), stop=(idx == 3 * KP - 1),
                                perf_mode=DR,
                            )
                            idx += 1
                    nc.scalar.mul(o_sb[:, oc * 256:(oc + 1) * 256], acc[:, :],
                                  1.0 / (SX * SM))
                    if ti == NT - 1:
                        # last tile gates the end of the kernel: DMA per chunk
                        nc.sync.dma_start(
                            out_d[ti * 128:(ti + 1) * 128,
                                  oc * 256:(oc + 1) * 256],
                            o_sb[:, oc * 256:(oc + 1) * 256])
                if ti < NT - 1:
                    nc.sync.dma_start(out_d[ti * 128:(ti + 1) * 128, :],
                                      o_sb[:, :])

    nc.compile()
    return nc


def _get_program():
    if "nc" not in _CACHE:
        _CACHE["nc"] = build_program()
    return _CACHE["nc"]


def kernel(x, kron_1, kron_2, kron_3, W):
    from concourse import bass_utils

    nc = _get_program()
    consts = _host_constants()
    x = np.asarray(x, dtype=np.float32)
    wt16 = np.ascontiguousarray(
        np.asarray(W, np.float32).T.reshape(2 * KP, 128, D)
        .transpose(1, 0, 2).astype(ml_dtypes.bfloat16))
    base = {
        "wt16": wt16,
        "npack": _newton_pack(np.asarray(kron_1, np.float32),
                              np.asarray(kron_2, np.float32),
                              np.asarray(kron_3, np.float32)),
        **consts,
    }
    in_maps = []
    for b in range(B):
        x8, xr8 = _pack_dr(np.ascontiguousarray(x[b].T), SX)
        in_maps.append({"x8": x8, "xr8": xr8, **base})
    res = bass_utils.run_bass_kernel_spmd(nc, in_maps, core_ids=list(range(B)))
    out = np.stack(
        [np.asarray(res.results[b]["out"], dtype=np.float32) for b in range(B)],
        axis=0,
    )
    return out.reshape(B, S, D)
